# revision 15
# baseline (speedup 1.0000x reference)
"""AttentionMIL pooling kernel for 8 Trainium2 NeuronCores.

Math (per slide b): h = tanh(X @ W1^T); s = h @ w2; a = softmax(s);
out = a^T @ X, with X [N=8192, D=1024], W1 [H=256, D], w2 [H].

Strategy (single-copy, ~33.6 MB/core HBM vs 67 MB for the two-layout
baseline):
  - Data-parallel over the slide dim: 16 slides / 8 cores = 2 per core.
  - Host ships ONLY the transposed bf16 layout xt (d on partitions, rows on
    free). Scores per 1024-row tile: ht = W1t-stationary @ xt on PE
    (32 matmuls of F=512 into a 4-bank fp32 PSUM tile), one tanh per
    h-chunk (ACT).
  - The w2 contraction is a PE matmul whose stationary is w2 REPLICATED
    across 128 columns: out [128, 1024] has every partition equal to the
    score row, i.e. the scores arrive already broadcast across partitions
    for the same cost (matmul cost depends only on F). exp on ACT fuses the
    PSUM->SBUF copy and emits accum_out = sum_j exp(s_j) (softmax
    denominator; no max-subtraction needed: |s| <= ||w2||_1 ~ 13 cannot
    overflow fp32).
  - Weighted sum WITHOUT a second X layout: per d-chunk k,
    r[p, k] += sum_j xt[p, k*1024+j] * e[j]. Chunks 0-4 via fused
    scalar_tensor_tensor on DVE (mul + free-dim reduce + [P,1] accum in one
    1x op). Chunks 5-7: one 2x tensor_tensor premultiply on DVE, then three
    activation-accumulate reduces on ACT. This splits the reduction so DVE
    (~117us), ACT (~116us) and PE (~124us) all stay near the ridge.
    Per-(chunk, tile) partials land in r_all slots; one tiny reduce per
    slide folds them; host divides by the denominator.
  - The score tail for tile t is emitted after the ht matmuls of tile t+1
    so the PE never stalls waiting on ACT; tanh is split per h-chunk so the
    single-buffer ht PSUM tile frees early for tile t+1.
"""

import sys

sys.path.insert(0, "/opt/trn_rl_repo")

import numpy as np
import ml_dtypes

import concourse.bacc as bacc
import concourse.tile as tile
from concourse import mybir
from concourse.bass_utils import run_bass_kernel_spmd

BF16 = ml_dtypes.bfloat16
B, N, D, H = 16, 8192, 1024, 256
NCORES = 8
SPC = B // NCORES          # slides per core
NT = 2048                  # rows of N per wsum super-group
TILES = N // NT
KCH = D // 128             # d-chunks (contraction blocks)
HCH = H // 128             # h-chunks
FH = NT // 512             # 512-wide F slices per tile (PSUM bank limit)
NDVE = 5                   # weighted-sum chunks via DVE STT; rest premul+ACT
SLOTS = TILES + 2          # partial-sum slots (first/last groups use two)

_NC_CACHE = {}


def _build_nc():
    bf = mybir.dt.bfloat16
    f32 = mybir.dt.float32
    AF = mybir.ActivationFunctionType
    ALU = mybir.AluOpType

    nc = bacc.Bacc("TRN2", num_devices=NCORES)
    # Host-swizzled: each per-tile DMA reads one contiguous 2 MiB region
    # into a [128, 8192] SBUF tile (128 descriptors x 16 KiB).
    #   xt[s, t, p, k*NT + j] = X[s, t*NT + j, k*128 + p]
    xt = nc.declare_dram_parameter("xt", [SPC, TILES, 128, KCH * NT], bf, isOutput=False)
    #   w1t[p, k*H + h] = W1[h, k*128 + p]
    w1t = nc.declare_dram_parameter("w1t", [128, KCH * H], bf, isOutput=False)
    #   w2rep[p, hc*128 + m] = W2[0, hc*128 + p]  (column-replicated blocks)
    w2rep = nc.declare_dram_parameter("w2rep", [128, HCH * 128], bf, isOutput=False)
    # out[s, p, k] = sum_n e_n X[n, k*128+p]  for k<8;  out[s, p, 8] = l
    outp = nc.declare_dram_parameter("out", [SPC, 128, KCH + 1], f32, isOutput=True)

    with tile.TileContext(nc) as tc:
        with tc.tile_pool(name="const", bufs=1) as constp, \
             tc.tile_pool(name="xt", bufs=3) as xtp, \
             tc.tile_pool(name="th", bufs=3) as thp, \
             tc.tile_pool(name="ebc", bufs=3) as ebcp, \
             tc.tile_pool(name="tmp3", bufs=2) as tmp3p, \
             tc.tile_pool(name="scr", bufs=1) as scrp, \
             tc.tile_pool(name="acc", bufs=1) as accp, \
             tc.tile_pool(name="osb", bufs=2) as osbp, \
             tc.tile_pool(name="htps", bufs=2, space="PSUM") as htpsp, \
             tc.tile_pool(name="warmps", bufs=1, space="PSUM") as warmpsp, \
             tc.tile_pool(name="sbc", bufs=2, space="PSUM") as sbcp:

            w1t_sb = constp.tile([128, KCH * H], bf)
            nc.gpsimd.dma_start(w1t_sb[:], w1t[:, :])
            w2rep_sb = constp.tile([128, HCH * 128], bf)
            nc.gpsimd.dma_start(w2rep_sb[:], w2rep[:, :])

            # p-state ramp: keep the PE busy ~3us so it reaches full clock
            # before the real matmuls (overlaps the first xt DMA).
            warm_sb = constp.tile([128, 256], bf)
            nc.gpsimd.memset(warm_sb[:], 0.0)
            warm_ps = warmpsp.tile([128, 512], f32)
            for _ in range(14):
                nc.tensor.matmul(
                    warm_ps[:, 0:256], warm_sb[:, 0:128], warm_sb[:, 0:256],
                    start=True, stop=True, skip_group_check=True,
                )

            # scratch for op outputs whose only consumed result is accum_out
            scr_dve = scrp.tile([128, NT], bf)
            scr_act = scrp.tile([128, NT], bf)

            for s in range(SPC):
                # per-(chunk, tile) weighted-sum partials + per-tile exp sums
                r_all = accp.tile([128, KCH * SLOTS], f32, tag=f"racc{s}")
                l_all = accp.tile([128, FH * TILES], f32, tag=f"lacc{s}")

                # score tail for one 512-row half: w2-matmul + exp
                def score_tail(u, f, th_sb, e_bc):
                    s_bc = sbcp.tile([128, 512], f32)
                    for hc in range(HCH):
                        nc.tensor.matmul(
                            s_bc[:],
                            w2rep_sb[:, hc * 128:(hc + 1) * 128],
                            th_sb[:, hc * 512:(hc + 1) * 512],
                            start=(hc == 0), stop=(hc == HCH - 1),
                        )
                    nc.scalar.activation(
                        e_bc[:, f * 512:(f + 1) * 512], s_bc[:], AF.Exp,
                        accum_out=l_all[:, FH * u + f:FH * u + f + 1],
                    )

                # weighted-sum ops for rows [j0, j1) of a super-group,
                # accumulating into partial-sum slot `slot` (0..SLOTS-1).
                # DVE work (premultiplies first so ACT's inputs are ready
                # early, then the fused mul+reduce STTs) is emitted here; the
                # ACT reduces go on a backlog flushed one half-iteration
                # later so tanh/exp keep queue priority on the scalar engine.
                act_backlog = []

                def wsum(slot, xt_sb, e_bc, j0, j1):
                    tmp3 = tmp3p.tile([128, (KCH - NDVE) * NT], bf, name="tmp3")
                    for i in range(KCH - NDVE):
                        nc.vector.tensor_mul(
                            tmp3[:, i * NT + j0:i * NT + j1],
                            xt_sb[:, (NDVE + i) * NT + j0:(NDVE + i) * NT + j1],
                            e_bc[:, j0:j1],
                        )
                    act_backlog.append((slot, tmp3, j0, j1))
                    for k in range(NDVE):
                        nc.vector.scalar_tensor_tensor(
                            out=scr_dve[:, 0:j1 - j0],
                            in0=xt_sb[:, k * NT + j0:k * NT + j1],
                            scalar=1.0,
                            in1=e_bc[:, j0:j1],
                            op0=ALU.mult,
                            op1=ALU.mult,
                            accum_out=r_all[:, k * SLOTS + slot:k * SLOTS + slot + 1],
                        )

                def flush_act():
                    for slot, tmp3, j0, j1 in act_backlog:
                        for i in range(KCH - NDVE):
                            k = NDVE + i
                            nc.scalar.activation(
                                scr_act[:, 0:j1 - j0],
                                tmp3[:, i * NT + j0:i * NT + j1],
                                AF.Copy,
                                accum_out=r_all[:, k * SLOTS + slot:k * SLOTS + slot + 1],
                            )
                    act_backlog.clear()

                pend_score = None   # (u, f, th_sb, e_bc) half awaiting tail
                pend_wsum = None    # (u, xt_sb, e_bc) group awaiting wsum
                e_bc = None
                last = TILES - 1
                for t in range(TILES):
                    xt_sb = xtp.tile([128, KCH * NT], bf)
                    for q in range(4):
                        qw = KCH * NT // 4
                        nc.sync.dma_start(
                            xt_sb[:, q * qw:(q + 1) * qw],
                            xt[s, t, :, q * qw:(q + 1) * qw],
                        )
                    e_bc = ebcp.tile([128, NT], bf, name="e_bc")
                    for f in range(FH):
                        ht_ps = htpsp.tile([128, HCH * 512], f32)
                        for hc in range(HCH):
                            for k in range(KCH):
                                nc.tensor.matmul(
                                    ht_ps[:, hc * 512:(hc + 1) * 512],
                                    w1t_sb[:, k * H + hc * 128: k * H + hc * 128 + 128],
                                    xt_sb[:, k * NT + f * 512: k * NT + f * 512 + 512],
                                    start=(k == 0), stop=(k == KCH - 1),
                                )
                        th_sb = thp.tile([128, HCH * 512], bf, name="th_sb")
                        nc.scalar.activation(th_sb[:], ht_ps[:], AF.Tanh)
                        if pend_score is not None:
                            score_tail(*pend_score)
                        pend_score = (t, f, th_sb, e_bc)
                        if pend_wsum is not None:
                            wsum(*pend_wsum)
                            pend_wsum = None
                        else:
                            flush_act()
                        if t == 0 and f == FH - 2:
                            # ramp shrink: first half of the first group can
                            # start as soon as its first two exps are done
                            wsum(TILES + 1, xt_sb, e_bc, 0, NT // 2)
                        if t == last and f == FH - 1:
                            # drain shrink: likewise for the final group
                            wsum(TILES - 1, xt_sb, e_bc, 0, NT // 2)
                    if t == 0:
                        pend_wsum = (0, xt_sb, e_bc, NT // 2, NT)
                    elif t < last:
                        pend_wsum = (t, xt_sb, e_bc, 0, NT)
                score_tail(*pend_score)
                flush_act()
                wsum(TILES, xt_sb, e_bc, NT // 2, NT)
                flush_act()

                o_sb = osbp.tile([128, KCH + 1], f32)
                nc.vector.reduce_sum(
                    o_sb[:, 0:KCH],
                    r_all[:].rearrange("p (k t) -> p k t", k=KCH),
                    axis=mybir.AxisListType.X,
                )
                nc.vector.reduce_sum(
                    o_sb[:, KCH:KCH + 1],
                    l_all[:].rearrange("p (o t) -> p o t", o=1),
                    axis=mybir.AxisListType.X,
                )
                nc.scalar.dma_start(outp[s], o_sb[:])

    nc.compile()
    return nc


def _get_nc():
    if "nc" not in _NC_CACHE:
        _NC_CACHE["nc"] = _build_nc()
    return _NC_CACHE["nc"]


def _prep_inputs(tiles_embeddings, W1, W2):
    X_bf = tiles_embeddings.astype(BF16)
    # xt[b, t, p, k, j] = X[b, t*NT + j, k*128 + p]
    xt_sw = np.ascontiguousarray(
        X_bf.reshape(B, TILES, NT, KCH, 128).transpose(0, 1, 4, 3, 2)
    ).reshape(B, TILES, 128, KCH * NT)
    # w1t[p, k, h] = W1[h, k*128 + p]
    w1t = np.ascontiguousarray(
        W1.astype(BF16).reshape(H, KCH, 128).transpose(2, 1, 0)
    ).reshape(128, KCH * H)
    # w2rep[p, hc*128 + m] = W2[0, hc*128 + p]
    w2c = W2.astype(BF16).reshape(HCH, 128)
    w2rep = np.ascontiguousarray(
        np.repeat(w2c[:, :, None], 128, axis=2).transpose(1, 0, 2)
    ).reshape(128, HCH * 128)
    return [
        {
            "xt": xt_sw[c * SPC:(c + 1) * SPC],
            "w1t": w1t,
            "w2rep": w2rep,
        }
        for c in range(NCORES)
    ]


def _run(tiles_embeddings, W1, W2, **spmd_kwargs):
    nc = _get_nc()
    in_maps = _prep_inputs(tiles_embeddings, W1, W2)
    res = run_bass_kernel_spmd(nc, in_maps, core_ids=list(range(NCORES)), **spmd_kwargs)
    raw = np.concatenate([r["out"] for r in res.results], axis=0)  # [B, 128, 9]
    acc = raw[:, :, 0:KCH].transpose(0, 2, 1).reshape(B, D)        # d = k*128 + p
    l = raw[:, 0, KCH]                                             # [B]
    out = acc / l[:, None]
    return out.astype(np.float32, copy=False), res


def kernel(tiles_embeddings, W1, W2):
    out, _ = _run(
        np.asarray(tiles_embeddings), np.asarray(W1), np.asarray(W2)
    )
    return out


# revision 17
# speedup vs baseline: 1.1705x; 1.1705x over previous
"""AttentionMIL pooling kernel for 8 Trainium2 NeuronCores.

Math (per slide b): h = tanh(X @ W1^T); s = h @ w2; a = softmax(s);
out = a^T @ X, with X [N=8192, D=1024], W1 [H=256, D], w2 [H].

Strategy (single-copy, ~33.6 MB/core HBM vs 67 MB for the two-layout
baseline; measured 221.8us -> ~171us):
  - Data-parallel over the slide dim: 16 slides / 8 cores = 2 per core.
  - Host ships ONLY the transposed bf16 layout xt (d on partitions, rows on
    free), as 2048-row super-groups DMA'd in four 1 MiB pieces so the PE can
    start on the first piece. Scores per 512-row half: ht = W1t-stationary
    @ xt on PE (16 matmuls of F=512 into a double-buffered 2-bank PSUM
    tile), one tanh per half (ACT).
  - The w2 contraction is a PE matmul whose stationary is w2 REPLICATED
    across 128 columns: out [128, 512] has every partition equal to the
    score row, i.e. the scores arrive already broadcast across partitions
    for the same cost (matmul cost depends only on F). exp on ACT fuses the
    PSUM->SBUF copy and emits accum_out = sum_j exp(s_j) (softmax
    denominator; no max-subtraction needed: |s| <= ||w2||_1 ~ 13 cannot
    overflow fp32).
  - Weighted sum WITHOUT a second X layout: per d-chunk k,
    r[p, k] += sum_j xt[p, k*2048+j] * e[j], at 2048-row granularity to
    amortize fixed per-op costs. Chunks 0-4 via fused scalar_tensor_tensor
    on DVE (mul + free-dim reduce + [P,1] accum in one 1x op). Chunks 5-7:
    2x tensor_tensor premultiplies on DVE, then three activation-accumulate
    reduces on ACT, emitted one half-iteration later so tanh/exp keep
    priority in the scalar engine's in-order queue. Measured steady state:
    DVE ~145us, ACT ~138us, PE ~145us busy — all near the ridge.
    Per-(chunk, group) partials land in r_all slots; one tiny reduce per
    slide folds them; host divides by the denominator.
  - The score tail for half f is emitted after the ht matmuls of half f+1
    so the PE never stalls waiting on ACT. The final group's weighted sum
    is split in two j-halves, the first emitted as soon as its two exps
    are done, shrinking the end-of-kernel drain. (Emitting wsum work any
    earlier than one group behind measurably regresses the schedule.)
"""

import sys

sys.path.insert(0, "/opt/trn_rl_repo")

import numpy as np
import ml_dtypes

import concourse.bacc as bacc
import concourse.tile as tile
from concourse import mybir
from concourse.bass_utils import run_bass_kernel_spmd

BF16 = ml_dtypes.bfloat16
B, N, D, H = 16, 8192, 1024, 256
NCORES = 8
SPC = B // NCORES          # slides per core
NT = 2048                  # rows of N per wsum super-group
TILES = N // NT
KCH = D // 128             # d-chunks (contraction blocks)
HCH = H // 128             # h-chunks
FH = NT // 512             # 512-wide F slices per tile (PSUM bank limit)
NDVE = 5                   # weighted-sum chunks via DVE STT; rest premul+ACT
SLOTS = TILES + 1          # partial-sum slots (last group uses two)

_NC_CACHE = {}


def _build_nc():
    bf = mybir.dt.bfloat16
    f32 = mybir.dt.float32
    AF = mybir.ActivationFunctionType
    ALU = mybir.AluOpType

    nc = bacc.Bacc("TRN2", num_devices=NCORES)
    # Host-swizzled: each per-tile DMA reads one contiguous 2 MiB region
    # into a [128, 8192] SBUF tile (128 descriptors x 16 KiB).
    #   xt[s, t, p, k*NT + j] = X[s, t*NT + j, k*128 + p]
    xt = nc.declare_dram_parameter("xt", [SPC, TILES, 128, KCH * NT], bf, isOutput=False)
    #   w1t[p, k*H + h] = W1[h, k*128 + p]
    w1t = nc.declare_dram_parameter("w1t", [128, KCH * H], bf, isOutput=False)
    #   w2rep[p, hc*128 + m] = W2[0, hc*128 + p]  (column-replicated blocks)
    w2rep = nc.declare_dram_parameter("w2rep", [128, HCH * 128], bf, isOutput=False)
    # out[s, p, k] = sum_n e_n X[n, k*128+p]  for k<8;  out[s, p, 8] = l
    outp = nc.declare_dram_parameter("out", [SPC, 128, KCH + 1], f32, isOutput=True)

    with tile.TileContext(nc) as tc:
        with tc.tile_pool(name="const", bufs=1) as constp, \
             tc.tile_pool(name="xt", bufs=3) as xtp, \
             tc.tile_pool(name="th", bufs=3) as thp, \
             tc.tile_pool(name="ebc", bufs=3) as ebcp, \
             tc.tile_pool(name="tmp3", bufs=2) as tmp3p, \
             tc.tile_pool(name="scr", bufs=1) as scrp, \
             tc.tile_pool(name="acc", bufs=1) as accp, \
             tc.tile_pool(name="osb", bufs=2) as osbp, \
             tc.tile_pool(name="htps", bufs=2, space="PSUM") as htpsp, \
             tc.tile_pool(name="warmps", bufs=1, space="PSUM") as warmpsp, \
             tc.tile_pool(name="sbc", bufs=2, space="PSUM") as sbcp:

            w1t_sb = constp.tile([128, KCH * H], bf)
            nc.gpsimd.dma_start(w1t_sb[:], w1t[:, :])
            w2rep_sb = constp.tile([128, HCH * 128], bf)
            nc.gpsimd.dma_start(w2rep_sb[:], w2rep[:, :])

            # p-state ramp: keep the PE busy ~3us so it reaches full clock
            # before the real matmuls (overlaps the first xt DMA).
            warm_sb = constp.tile([128, 256], bf)
            nc.gpsimd.memset(warm_sb[:], 0.0)
            warm_ps = warmpsp.tile([128, 512], f32)
            for _ in range(14):
                nc.tensor.matmul(
                    warm_ps[:, 0:256], warm_sb[:, 0:128], warm_sb[:, 0:256],
                    start=True, stop=True, skip_group_check=True,
                )

            # scratch for op outputs whose only consumed result is accum_out
            scr_dve = scrp.tile([128, NT], bf)
            scr_act = scrp.tile([128, NT], bf)

            for s in range(SPC):
                # per-(chunk, tile) weighted-sum partials + per-tile exp sums
                r_all = accp.tile([128, KCH * SLOTS], f32, tag=f"racc{s}")
                l_all = accp.tile([128, FH * TILES], f32, tag=f"lacc{s}")

                # score tail for one 512-row half: w2-matmul + exp
                def score_tail(u, f, th_sb, e_bc):
                    s_bc = sbcp.tile([128, 512], f32)
                    for hc in range(HCH):
                        nc.tensor.matmul(
                            s_bc[:],
                            w2rep_sb[:, hc * 128:(hc + 1) * 128],
                            th_sb[:, hc * 512:(hc + 1) * 512],
                            start=(hc == 0), stop=(hc == HCH - 1),
                        )
                    nc.scalar.activation(
                        e_bc[:, f * 512:(f + 1) * 512], s_bc[:], AF.Exp,
                        accum_out=l_all[:, FH * u + f:FH * u + f + 1],
                    )

                # weighted-sum ops for rows [j0, j1) of a super-group,
                # accumulating into partial-sum slot `slot` (0..SLOTS-1).
                # DVE work (premultiplies first so ACT's inputs are ready
                # early, then the fused mul+reduce STTs) is emitted here; the
                # ACT reduces go on a backlog flushed one half-iteration
                # later so tanh/exp keep queue priority on the scalar engine.
                act_backlog = []

                def wsum(slot, xt_sb, e_bc, j0, j1):
                    tmp3 = tmp3p.tile([128, (KCH - NDVE) * NT], bf, name="tmp3")
                    for i in range(KCH - NDVE):
                        nc.vector.tensor_mul(
                            tmp3[:, i * NT + j0:i * NT + j1],
                            xt_sb[:, (NDVE + i) * NT + j0:(NDVE + i) * NT + j1],
                            e_bc[:, j0:j1],
                        )
                    act_backlog.append((slot, tmp3, j0, j1))
                    for k in range(NDVE):
                        nc.vector.scalar_tensor_tensor(
                            out=scr_dve[:, 0:j1 - j0],
                            in0=xt_sb[:, k * NT + j0:k * NT + j1],
                            scalar=1.0,
                            in1=e_bc[:, j0:j1],
                            op0=ALU.mult,
                            op1=ALU.mult,
                            accum_out=r_all[:, k * SLOTS + slot:k * SLOTS + slot + 1],
                        )

                def flush_act():
                    for slot, tmp3, j0, j1 in act_backlog:
                        for i in range(KCH - NDVE):
                            k = NDVE + i
                            nc.scalar.activation(
                                scr_act[:, 0:j1 - j0],
                                tmp3[:, i * NT + j0:i * NT + j1],
                                AF.Copy,
                                accum_out=r_all[:, k * SLOTS + slot:k * SLOTS + slot + 1],
                            )
                    act_backlog.clear()

                pend_score = None   # (u, f, th_sb, e_bc) half awaiting tail
                pend_wsum = None    # (u, xt_sb, e_bc) group awaiting wsum
                e_bc = None
                last = TILES - 1
                for t in range(TILES):
                    xt_sb = xtp.tile([128, KCH * NT], bf)
                    for q in range(4):
                        qw = KCH * NT // 4
                        nc.sync.dma_start(
                            xt_sb[:, q * qw:(q + 1) * qw],
                            xt[s, t, :, q * qw:(q + 1) * qw],
                        )
                    e_bc = ebcp.tile([128, NT], bf, name="e_bc")
                    for f in range(FH):
                        ht_ps = htpsp.tile([128, HCH * 512], f32)
                        for hc in range(HCH):
                            for k in range(KCH):
                                nc.tensor.matmul(
                                    ht_ps[:, hc * 512:(hc + 1) * 512],
                                    w1t_sb[:, k * H + hc * 128: k * H + hc * 128 + 128],
                                    xt_sb[:, k * NT + f * 512: k * NT + f * 512 + 512],
                                    start=(k == 0), stop=(k == KCH - 1),
                                )
                        th_sb = thp.tile([128, HCH * 512], bf, name="th_sb")
                        nc.scalar.activation(th_sb[:], ht_ps[:], AF.Tanh)
                        if pend_score is not None:
                            score_tail(*pend_score)
                        pend_score = (t, f, th_sb, e_bc)
                        if pend_wsum is not None:
                            wsum(*pend_wsum)
                            pend_wsum = None
                        else:
                            flush_act()
                        if t == last and f == FH - 1:
                            # drain shrink: first half of the final group can
                            # start as soon as its first two exps are done
                            wsum(TILES - 1, xt_sb, e_bc, 0, NT // 2)
                    if t < last:
                        pend_wsum = (t, xt_sb, e_bc, 0, NT)
                score_tail(*pend_score)
                flush_act()
                wsum(TILES, xt_sb, e_bc, NT // 2, NT)
                flush_act()

                o_sb = osbp.tile([128, KCH + 1], f32)
                nc.vector.reduce_sum(
                    o_sb[:, 0:KCH],
                    r_all[:].rearrange("p (k t) -> p k t", k=KCH),
                    axis=mybir.AxisListType.X,
                )
                nc.vector.reduce_sum(
                    o_sb[:, KCH:KCH + 1],
                    l_all[:].rearrange("p (o t) -> p o t", o=1),
                    axis=mybir.AxisListType.X,
                )
                nc.scalar.dma_start(outp[s], o_sb[:])

    nc.compile()
    return nc


def _get_nc():
    if "nc" not in _NC_CACHE:
        _NC_CACHE["nc"] = _build_nc()
    return _NC_CACHE["nc"]


def _prep_inputs(tiles_embeddings, W1, W2):
    X_bf = tiles_embeddings.astype(BF16)
    # xt[b, t, p, k, j] = X[b, t*NT + j, k*128 + p]
    xt_sw = np.ascontiguousarray(
        X_bf.reshape(B, TILES, NT, KCH, 128).transpose(0, 1, 4, 3, 2)
    ).reshape(B, TILES, 128, KCH * NT)
    # w1t[p, k, h] = W1[h, k*128 + p]
    w1t = np.ascontiguousarray(
        W1.astype(BF16).reshape(H, KCH, 128).transpose(2, 1, 0)
    ).reshape(128, KCH * H)
    # w2rep[p, hc*128 + m] = W2[0, hc*128 + p]
    w2c = W2.astype(BF16).reshape(HCH, 128)
    w2rep = np.ascontiguousarray(
        np.repeat(w2c[:, :, None], 128, axis=2).transpose(1, 0, 2)
    ).reshape(128, HCH * 128)
    return [
        {
            "xt": xt_sw[c * SPC:(c + 1) * SPC],
            "w1t": w1t,
            "w2rep": w2rep,
        }
        for c in range(NCORES)
    ]


def _run(tiles_embeddings, W1, W2, **spmd_kwargs):
    nc = _get_nc()
    in_maps = _prep_inputs(tiles_embeddings, W1, W2)
    res = run_bass_kernel_spmd(nc, in_maps, core_ids=list(range(NCORES)), **spmd_kwargs)
    raw = np.concatenate([r["out"] for r in res.results], axis=0)  # [B, 128, 9]
    acc = raw[:, :, 0:KCH].transpose(0, 2, 1).reshape(B, D)        # d = k*128 + p
    l = raw[:, 0, KCH]                                             # [B]
    out = acc / l[:, None]
    return out.astype(np.float32, copy=False), res


def kernel(tiles_embeddings, W1, W2):
    out, _ = _run(
        np.asarray(tiles_embeddings), np.asarray(W1), np.asarray(W2)
    )
    return out
